# revision 10
# baseline (speedup 1.0000x reference)
"""GQA kernel for Trainium2, 8 NeuronCores.

Problem: B=2, T=2048, HIDDEN=1024, 16 q-heads, 4 kv-heads, head_dim=64,
causal attention + output projection.

Sharding: core = (batch b = core//4, kv-group g = core%4). Each core handles
one batch element and the 4 query heads sharing kv-head g. o_proj is
column-parallel after per-chunk AllGathers (bf16) of the normalized attention
outputs within each batch group of 4 cores.

Device dataflow (all matmuls bf16 with fp32 PSUM accumulation):
  - host supplies xT = x[b].T in bf16 ([1024, 2048]; hidden on partitions)
  - qT/kT via W-stationary matmuls (outputs transposed: head_dim on partitions)
  - V natural via PE transposes of vT tiles; ones column appended -> softmax
    denominators fall out of the PV matmul for free
  - S^T = kT.T @ qT directly (no transposes in the attention inner loop);
    2 heads packed per pass via PE row-tiling (K=64 each)
  - exp on ACT engine; causal mask = bf16 0/1 multiply on diagonal tiles only
  - o^T_aug[65, Tq] accumulated per head in PSUM, evacuated to SBUF fast
    (frees PSUM for the next chunk); normalization trails off-path
  - per-chunk AllGather of normalized attn^T (bf16) -> col-parallel o_proj
  - output is outT [256, 2048] (columns 256g..256g+256 of out[b], transposed);
    host concatenates and transposes back.
"""

import sys

import numpy as np

try:
    import concourse.bass as bass
except ImportError:
    sys.path.insert(0, "/opt/trn_rl_repo")
    import concourse.bass as bass

import ml_dtypes
from contextlib import ExitStack

import concourse.tile as tile
from concourse import bacc, mybir
from concourse.bass import ds, ts
from concourse.bass_utils import run_bass_kernel_spmd
from concourse.masks import make_identity

BF16 = mybir.dt.bfloat16
F32 = mybir.dt.float32

P = 128
T = 2048
HID = 1024
KT = HID // P  # 8 k-tiles over hidden
CH = 512       # T_q chunk width
NCHUNK = T // CH
D = 64         # head dim
SCALE = D ** -0.5

_PROGRAM = None


def build_program():
    nc = bacc.Bacc(num_devices=8)

    xT_d = nc.declare_dram_parameter("xT", [HID, T], BF16, isOutput=False)
    wqkv_d = nc.declare_dram_parameter("wqkv", [HID, 384], BF16, isOutput=False)
    wo_d = nc.declare_dram_parameter("wo", [HID, 256], BF16, isOutput=False)
    mask_d = nc.declare_dram_parameter("maskc", [P, 1024], BF16, isOutput=False)
    outT_d = nc.declare_dram_parameter("outT", [256, T], F32, isOutput=True)

    with tile.TileContext(nc) as tc, ExitStack() as ctx:
        sing = ctx.enter_context(tc.tile_pool(name="sing", bufs=1))
        work = ctx.enter_context(tc.tile_pool(name="work", bufs=2, space="PSUM"))
        accp = ctx.enter_context(tc.tile_pool(name="accp", bufs=4, space="PSUM"))
        ptp = ctx.enter_context(tc.tile_pool(name="ptp", bufs=4))
        outp = ctx.enter_context(tc.tile_pool(name="outp", bufs=3))
        nrmp = ctx.enter_context(tc.tile_pool(name="nrmp", bufs=8))
        oevp = ctx.enter_context(tc.tile_pool(name="oevp", bufs=8))
        agp = ctx.enter_context(tc.tile_pool(name="agp", bufs=2))
        dram = ctx.enter_context(tc.tile_pool(name="dram", bufs=1, space="DRAM"))

        AGW = [1024, 512, 512]          # columns per gather
        AGO = [0, 1024, 1536]           # column offset of each gather in T
        ag_in = [
            dram.tile([256, w], BF16, name=f"ag_in{i}")
            for i, w in enumerate(AGW)
        ]
        ag_out = [
            dram.tile([4 * 256, w], BF16, name=f"ag_out{i}")
            for i, w in enumerate(AGW)
        ]

        # --- loads needed before chunk-0 compute: wqkv, xT chunk 0, mask ---
        wqkv_sb = sing.tile([P, KT, 384], BF16)
        nc.sync.dma_start(wqkv_sb, wqkv_d[:, :].rearrange("(kt p) n -> p kt n", p=P))
        xT_sb = sing.tile([P, KT, T], BF16)
        for kt in range(KT):
            nc.sync.dma_start(xT_sb[:, kt, ts(0, CH)], xT_d[ts(kt, P), ts(0, CH)])
        maskc = sing.tile([P, 1024], BF16)
        nc.sync.dma_start(maskc, mask_d[:, :])
        ident = sing.tile([P, P], BF16)
        make_identity(nc, ident)
        # --- deferred loads ---
        for c in range(1, NCHUNK):
            for kt in range(KT):
                nc.sync.dma_start(xT_sb[:, kt, ts(c, CH)], xT_d[ts(kt, P), ts(c, CH)])
        wo_sb = sing.tile([P, KT, 256], BF16)
        nc.sync.dma_start(wo_sb, wo_d[:, :].rearrange("(kt p) n -> p kt n", p=P))

        # blocks: 0 = qT heads (0,1); 1 = qT heads (2,3); 2 = [kT | vT]
        qkvT_sb = sing.tile([P, 3, T], BF16)
        kdup = sing.tile([P, T], BF16)        # kT duplicated on both partition halves
        vaug = sing.tile([P, 16, 66], BF16)   # V natural per T_k tile + ones col (64)
        nc.gpsimd.memset(vaug[:, :, 64:65], 1.0)
        agT = sing.tile([P, KT, T], BF16)     # gathered attn^T for o_proj

        for c in range(NCHUNK):
            cs = ts(c, CH)
            # ---- qkv projection for this T-chunk ----
            for blk in range(3):
                pj = work.tile([P, 1024], F32, tag="work", name=f"pj{c}_{blk}")
                for kt in range(KT):
                    nc.tensor.matmul(
                        pj[:, 0:CH],
                        wqkv_sb[:, kt, ts(blk, P)],
                        xT_sb[:, kt, cs],
                        start=(kt == 0),
                        stop=(kt == KT - 1),
                    )
                if blk < 2:
                    nc.vector.tensor_copy(qkvT_sb[:, blk, cs], pj[:, 0:CH])
                else:
                    nc.vector.tensor_copy(kdup[0:64, cs], pj[0:64, 0:CH])
                    nc.vector.tensor_copy(kdup[64:128, cs], pj[0:64, 0:CH])
                    nc.vector.tensor_copy(qkvT_sb[64:128, 2, cs], pj[64:128, 0:CH])

            # ---- V natural for the 4 new T_k tiles ----
            for j in range(4 * c, 4 * c + 4):
                vps = work.tile([P, 64], BF16, tag="work", name=f"vps{j}")
                nc.tensor.transpose(
                    vps[:, 0:64], qkvT_sb[64:128, 2, ts(j, P)], ident[64:128, 64:128]
                )
                nc.vector.tensor_copy(vaug[:, j, 0:64], vps[:, 0:64])

            # ---- attention for chunk c ----
            ntk = 4 * (c + 1)
            oa = [
                accp.tile([P, CH], F32, tag="acc", name=f"oa{c}_{h}")
                for h in range(4)
            ]
            for j in range(ntk):
                diag = j >= 4 * c
                d_off = P * (j - 4 * c)
                for hp in range(2):
                    s2 = work.tile([P, 1024], F32, tag="work", name=f"s2_{c}_{j}_{hp}")
                    nc.tensor.matmul(
                        s2[:, 0:CH],
                        kdup[0:64, ts(j, P)],
                        qkvT_sb[0:64, hp, cs],
                        start=True,
                        stop=True,
                        tile_position=(0, 0),
                    )
                    nc.tensor.matmul(
                        s2[:, CH:1024],
                        kdup[64:128, ts(j, P)],
                        qkvT_sb[64:128, hp, cs],
                        start=True,
                        stop=True,
                        tile_position=(64, 0),
                    )
                    pt = ptp.tile([P, 1024], BF16, tag="pt", name=f"pt{c}_{j}_{hp}")
                    nc.scalar.activation(pt, s2, mybir.ActivationFunctionType.Exp)
                    if diag:
                        msl = maskc[:, ds(384 - d_off, CH)]
                        nc.vector.tensor_mul(pt[:, 0:CH], pt[:, 0:CH], msl)
                        nc.vector.tensor_mul(pt[:, CH:1024], pt[:, CH:1024], msl)
                    for hh in range(2):
                        h = 2 * hp + hh
                        nc.tensor.matmul(
                            oa[h][0:65, :],
                            vaug[:, j, 0:65],
                            pt[:, ts(hh, CH)],
                            start=(j == 0),
                            stop=(j == ntk - 1),
                        )

            # ---- evacuate accumulators to SBUF fast (frees PSUM) ----
            # den rows land at 32-aligned partitions of one tile so a single
            # DVE reciprocal serves all 4 heads (cost is free-dim-only).
            den128 = nrmp.tile([P, CH], F32, tag="den", name=f"den{c}")
            oev = []
            for h in range(4):
                oe = oevp.tile([64, CH], F32, tag="oev", name=f"oev{c}_{h}")
                nc.vector.tensor_copy(oe, oa[h][0:64, :])
                nc.vector.tensor_copy(
                    den128[ds(32 * h, 1), :], oa[h][64:65, :]
                )
                oev.append(oe)

            # ---- normalize (off critical path) + stage + ship chunk ----
            atst = agp.tile([P, 2, CH], BF16, tag="atst", name=f"atst{c}")
            rcp128 = nrmp.tile([P, CH], F32, tag="rcp", name=f"rcp{c}")
            nc.vector.reciprocal(rcp128, den128)
            for h in range(4):
                rch = nrmp.tile([1, CH], F32, tag="rch", name=f"rch{c}_{h}")
                nc.vector.tensor_copy(rch, rcp128[ds(32 * h, 1), :])
                rb = nrmp.tile([64, CH], F32, tag="rb", name=f"rb{c}_{h}")
                nc.gpsimd.partition_broadcast(rb, rch)
                nc.vector.tensor_mul(
                    atst[ds(64 * (h % 2), 64), h // 2, :], oev[h], rb
                )
            ag = 0 if c < 2 else c - 1
            agv = ag_in[ag].rearrange("(blk p) t -> p blk t", p=P)
            nc.sync.dma_start(
                agv[:, :, ds(c * CH - AGO[ag], CH)], atst
            )

            if c != 0:
                # ---- AllGather within the batch group ----
                # (gathers 0-1 hide under later attention; 2 is the tail)
                nc.gpsimd.collective_compute(
                    "AllGather",
                    mybir.AluOpType.bypass,
                    replica_groups=[[0, 1, 2, 3], [4, 5, 6, 7]],
                    ins=[ag_in[ag].opt()],
                    outs=[ag_out[ag].opt()],
                )
                # ---- col-parallel o_proj for the gathered span ----
                for kt in range(KT):
                    nc.sync.dma_start(
                        agT[:, kt, ds(AGO[ag], AGW[ag])],
                        ag_out[ag][ts(kt, P), :],
                    )
                for c2 in range(AGO[ag] // CH, (AGO[ag] + AGW[ag]) // CH):
                    for mb in range(2):
                        ps = work.tile(
                            [P, 1024], F32, tag="work", name=f"ps{c2}_{mb}"
                        )
                        for kt in range(KT):
                            nc.tensor.matmul(
                                ps[:, 0:CH],
                                wo_sb[:, kt, ts(mb, P)],
                                agT[:, kt, ts(c2, CH)],
                                start=(kt == 0),
                                stop=(kt == KT - 1),
                            )
                        ob = outp.tile([P, CH], F32, tag="ob", name=f"ob{c2}_{mb}")
                        nc.vector.tensor_copy(ob, ps[:, 0:CH])
                        nc.sync.dma_start(outT_d[ts(mb, P), ts(c2, CH)], ob)

    nc.finalize()
    return nc


def _prep_inputs(x, Wq, Wkv, Wo):
    bf = ml_dtypes.bfloat16
    x = np.asarray(x, dtype=np.float32)
    Wq = np.asarray(Wq, dtype=np.float32)
    Wkv = np.asarray(Wkv, dtype=np.float32)
    Wo = np.asarray(Wo, dtype=np.float32)

    # causal mask bank: M[r, m] = 1.0 iff r <= m - 384 (else 0)
    mask = (np.arange(P)[:, None] <= (np.arange(1024)[None, :] - 384)).astype(bf)

    xT = [np.ascontiguousarray(x[b].T).astype(bf) for b in range(2)]

    in_maps = []
    for core in range(8):
        b, g = core // 4, core % 4
        wq_g = Wq[:, 256 * g : 256 * (g + 1)] * SCALE
        wk_g = Wkv[:, 64 * g : 64 * (g + 1)]
        wv_g = Wkv[:, 256 + 64 * g : 256 + 64 * (g + 1)]
        wqkv = np.ascontiguousarray(
            np.concatenate([wq_g, wk_g, wv_g], axis=1)
        ).astype(bf)
        wo_g = np.ascontiguousarray(Wo[:, 256 * g : 256 * (g + 1)]).astype(bf)
        in_maps.append(
            {"xT": xT[b], "wqkv": wqkv, "wo": wo_g, "maskc": mask}
        )
    return in_maps


def run(x, Wq, Wkv, Wo, trace=False, **trace_kwargs):
    global _PROGRAM
    if _PROGRAM is None:
        _PROGRAM = build_program()
    nc = _PROGRAM
    in_maps = _prep_inputs(x, Wq, Wkv, Wo)
    res = run_bass_kernel_spmd(
        nc, in_maps, core_ids=list(range(8)), trace=trace, **trace_kwargs
    )
    outs = res.results
    full = np.empty((2, T, HID), dtype=np.float32)
    for b in range(2):
        outT_b = np.concatenate(
            [np.asarray(outs[4 * b + g]["outT"]) for g in range(4)], axis=0
        )  # [1024, 2048]
        full[b] = outT_b.T
    return full, res


def kernel(x, Wq, Wkv, Wo):
    out, _ = run(x, Wq, Wkv, Wo, trace=False)
    return out



# revision 12
# speedup vs baseline: 1.5538x; 1.5538x over previous
"""GQA kernel for Trainium2, 8 NeuronCores — query-sharded, collective-free.

Problem: B=2, T=2048, HIDDEN=1024, 16 q-heads, 4 kv-heads, head_dim=64,
causal attention + output projection.

Sharding: core = (batch b = core//4, q-interleave g = core%4). Each core
handles ALL 16 heads for q-tiles {4c+g : c=0..3} (4 x 128 queries). KV is
computed redundantly on every core of a batch group, so qkv, attention AND
o_proj are fully local: no collectives at all (collective latency in this
environment is large and wildly variable).

Causal balance: chunk-slot c processes q-tile 4c+g against key tiles
0..4c+3 (uniform program). Key tiles beyond the causal limit (jd > g) are
killed via a per-partition bias of -30 fed to the EXP activation
(exp(s-30) ~ 0); the diagonal tile (jd == g) gets a triangular 0/1
mask-multiply (per-core mask data selects triangle vs all-ones so the
instruction stream stays uniform across cores).

Device dataflow (all matmuls bf16, fp32 PSUM):
  - q-proj: col-tiled head pairs (M=64+64) over xq (own queries, compact)
    -> qT [64d, head, 512q]
  - kT via W-stationary matmuls ([g0|g1] and [g2|g3] partition-stacked,
    which row-packs directly into the score matmuls)
  - V natural directly: x_tile.T @ Wv -> [128 keys, 256 vdims]; ones col
    appended -> softmax denominators fall out of the PV matmul
  - scores: per (j, group-half): 2 row-packed matmuls K=64 -> s2 [128k,
    2 groups x 4 heads x 128q]; EXP on ACT with causal bias; PV per group
  - normalize: denominator rows collected at 32-aligned partitions, ONE
    DVE reciprocal per chunk, gpsimd broadcast, per-head muls into attnT
  - o_proj local: Wo.T @ attnT per slot; slots 0-1 batched (N=256) after
    chunk-2 attention, slot 2 after chunk-3, slot 3 is the only tail.
  - output outT [1024, 512 own q] f32; host re-interleaves.
"""

import sys

import numpy as np

try:
    import concourse.bass as bass
except ImportError:
    sys.path.insert(0, "/opt/trn_rl_repo")
    import concourse.bass as bass

import ml_dtypes
from contextlib import ExitStack

import concourse.tile as tile
from concourse import bacc, mybir
from concourse.bass import ds, ts
from concourse.bass_utils import run_bass_kernel_spmd

BF16 = mybir.dt.bfloat16
F32 = mybir.dt.float32

P = 128
T = 2048
HID = 1024
KT = HID // P   # 8 k-tiles over hidden
CH = 512        # q columns per core (4 tiles of 128)
D = 64
SCALE = D ** -0.5
NEG = -30.0

_PROGRAM = None


def build_program():
    nc = bacc.Bacc(num_devices=8)

    xT_d = nc.declare_dram_parameter("xT", [HID, T], BF16, isOutput=False)
    xq_d = nc.declare_dram_parameter("xq", [HID, CH], BF16, isOutput=False)
    wq_d = nc.declare_dram_parameter("wq", [HID, 1024], BF16, isOutput=False)
    wkv_d = nc.declare_dram_parameter("wkv", [HID, 512], BF16, isOutput=False)
    wo_d = nc.declare_dram_parameter("wo", [HID, 1024], BF16, isOutput=False)
    mask_d = nc.declare_dram_parameter("maskq", [P, 4 * 1024], BF16, isOutput=False)
    bias_d = nc.declare_dram_parameter("biasb", [P, 4], F32, isOutput=False)
    outT_d = nc.declare_dram_parameter("outT", [HID, CH], BF16, isOutput=True)

    with tile.TileContext(nc) as tc, ExitStack() as ctx:
        sing = ctx.enter_context(tc.tile_pool(name="sing", bufs=1))
        work = ctx.enter_context(tc.tile_pool(name="work", bufs=2, space="PSUM"))
        accp = ctx.enter_context(tc.tile_pool(name="accp", bufs=4, space="PSUM"))
        ptp = ctx.enter_context(tc.tile_pool(name="ptp", bufs=4))
        outp = ctx.enter_context(tc.tile_pool(name="outp", bufs=3))
        nrmp = ctx.enter_context(tc.tile_pool(name="nrmp", bufs=2))
        oevp = ctx.enter_context(tc.tile_pool(name="oevp", bufs=6))

        # --- loads needed before chunk-0 compute ---
        xq_sb = sing.tile([P, KT, CH], BF16)
        nc.sync.dma_start(xq_sb, xq_d[:, :].rearrange("(kt p) n -> p kt n", p=P))
        wkv_sb = sing.tile([P, KT, 512], BF16)
        nc.sync.dma_start(wkv_sb, wkv_d[:, :].rearrange("(kt p) n -> p kt n", p=P))
        wq_sb = sing.tile([P, KT, 1024], BF16)
        nc.sync.dma_start(
            wq_sb[:, :, 0:512],
            wq_d[:, 0:512].rearrange("(kt p) n -> p kt n", p=P),
        )
        xT_sb = sing.tile([P, KT, T], BF16)
        for kt in range(KT):
            nc.sync.dma_start(xT_sb[:, kt, ts(0, CH)], xT_d[ts(kt, P), ts(0, CH)])
        nc.sync.dma_start(
            wq_sb[:, :, 512:1024],
            wq_d[:, 512:1024].rearrange("(kt p) n -> p kt n", p=P),
        )
        maskq = sing.tile([P, 4, 1024], BF16)
        nc.sync.dma_start(
            maskq, mask_d[:, :].rearrange("p (v n) -> p v n", v=4)
        )
        biasb = sing.tile([P, 4], F32)
        nc.sync.dma_start(biasb, bias_d[:, :])
        # --- deferred loads ---
        for c in range(1, 4):
            for kt in range(KT):
                nc.sync.dma_start(
                    xT_sb[:, kt, ts(c, CH)], xT_d[ts(kt, P), ts(c, CH)]
                )
        wo_sb = sing.tile([P, KT, 1024], BF16)
        nc.sync.dma_start(wo_sb, wo_d[:, :].rearrange("(kt p) n -> p kt n", p=P))

        qT_sb = sing.tile([P, 16, CH], BF16)      # [d dup'd on both halves, head, own q]
        kT_sb = sing.tile([P, 2, T], BF16)        # [g-pair dims, block, keys]
        vaug = sing.tile([P, 16, 4, 66], BF16)    # [keys, tile, group, 64+1]
        nc.gpsimd.memset(vaug[:, :, :, 64:65], 1.0)
        attnT = sing.tile([P, KT, CH], BF16)      # normalized attn^T for o_proj

        # ---- q-proj: all 4 slots ----
        def emit_qproj(mhs):
            for mh in mhs:
                qp = work.tile([P, CH], F32, tag="work", name=f"qp{mh}")
                for kt in range(KT):
                    nc.tensor.matmul(
                        qp,
                        wq_sb[:, kt, ts(mh, P)],
                        xq_sb[:, kt, :],
                        start=(kt == 0),
                        stop=(kt == KT - 1),
                    )
                nc.vector.tensor_copy(qT_sb[0:64, 2 * mh, :], qp[0:64, :])
                nc.vector.tensor_copy(qT_sb[64:128, 2 * mh + 1, :], qp[64:128, :])
                nc.sync.dma_start(
                    qT_sb[64:128, 2 * mh, :], qT_sb[0:64, 2 * mh, :]
                )
                nc.sync.dma_start(
                    qT_sb[0:64, 2 * mh + 1, :], qT_sb[64:128, 2 * mh + 1, :]
                )

        def emit_oproj(c0, nsl):
            # local o_proj for slots [c0, c0+nsl)
            w = 128 * nsl
            for mb in range(8):
                op = work.tile([P, CH], F32, tag="work", name=f"op{c0}_{mb}")
                for kt in range(KT):
                    nc.tensor.matmul(
                        op[:, 0:w],
                        wo_sb[:, kt, ts(mb, P)],
                        attnT[:, kt, ds(128 * c0, w)],
                        start=(kt == 0),
                        stop=(kt == KT - 1),
                    )
                ob = outp.tile([P, w], BF16, tag="ob", name=f"ob{c0}_{mb}")
                nc.vector.tensor_copy(ob, op[:, 0:w])
                nc.sync.dma_start(outT_d[ts(mb, P), ds(128 * c0, w)], ob)

        def emit_kv(c):
            # kv-proj for key chunk c (keys 512c .. 512c+511)
            for mb in range(2):
                kp = work.tile([P, CH], F32, tag="work", name=f"kp{c}_{mb}")
                for kt in range(KT):
                    nc.tensor.matmul(
                        kp,
                        wkv_sb[:, kt, ts(mb, P)],
                        xT_sb[:, kt, ts(c, CH)],
                        start=(kt == 0),
                        stop=(kt == KT - 1),
                    )
                nc.vector.tensor_copy(kT_sb[:, mb, ts(c, CH)], kp)
            for jt in range(4):
                j = 4 * c + jt
                vp = work.tile([P, 256], F32, tag="work", name=f"vp{j}")
                for kt in range(KT):
                    nc.tensor.matmul(
                        vp,
                        xT_sb[:, kt, ts(j, P)],
                        wkv_sb[:, kt, ds(256, 256)],
                        start=(kt == 0),
                        stop=(kt == KT - 1),
                    )
                nc.vector.tensor_copy(vaug[:, j, :, 0:64], vp)

        emit_qproj(range(8))

        # Slot order 0,1,3,2: the LAST chunk processed is the short one
        # (12 key tiles), shrinking the serial tail (norm + o_proj). kv
        # chunks 2 and 3 are produced inside slot 3's long early j-loop.
        # Old key tiles (j < 4c) don't need this chunk's kv, so each
        # j-loop starts immediately at the chunk boundary and the kv-proj
        # matmuls hide under the ACT-bound early iterations.
        KV_AT = {0: {0: 0}, 1: {4: 1}, 2: {8: 2}, 3: {12: 3}}
        for c in (0, 1, 2, 3):
            # ---- attention for slot c (q-tile 4c+g, 128 queries) ----
            ntk = 4 * (c + 1)
            oa = [
                accp.tile([P, CH], F32, tag="acc", name=f"oa{c}_{gr}")
                for gr in range(4)
            ]
            for j in range(ntk):
                jd = j - 4 * c  # >= 0 -> diagonal region
                if j in KV_AT[c]:
                    emit_kv(KV_AT[c][j])
                for gh in range(2):
                    s2 = work.tile(
                        [P, 1024], F32, tag="work", name=f"s2_{c}_{j}_{gh}"
                    )
                    nc.tensor.matmul(
                        s2[:, 0:CH],
                        kT_sb[0:64, gh, ts(j, P)],
                        qT_sb[0:64, ds(8 * gh, 4), ds(128 * c, P)],
                        start=True,
                        stop=True,
                        tile_position=(0, 0),
                    )
                    nc.tensor.matmul(
                        s2[:, CH:1024],
                        kT_sb[64:128, gh, ts(j, P)],
                        qT_sb[64:128, ds(8 * gh + 4, 4), ds(128 * c, P)],
                        start=True,
                        stop=True,
                        tile_position=(64, 0),
                    )
                    pt = ptp.tile([P, 1024], BF16, tag="pt", name=f"pt{c}_{j}_{gh}")
                    if jd >= 0:
                        # bias kills fully-masked tiles (jd > g): exp(s-30)~0
                        nc.scalar.activation(
                            pt, s2, mybir.ActivationFunctionType.Exp,
                            bias=biasb[:, jd : jd + 1],
                        )
                        # triangle on the diagonal tile (mask data is
                        # all-ones on cores where jd != g)
                        nc.vector.tensor_mul(pt, pt, maskq[:, jd, :])
                    else:
                        nc.scalar.activation(
                            pt, s2, mybir.ActivationFunctionType.Exp
                        )
                    for gg in range(2):
                        gr = 2 * gh + gg
                        nc.tensor.matmul(
                            oa[gr][0:65, :],
                            vaug[:, j, gr, 0:65],
                            pt[:, ts(gg, CH)],
                            start=(j == 0),
                            stop=(j == ntk - 1),
                        )

            # o_proj for ALREADY-normalized earlier slots goes here, right
            # after this chunk's attention matmuls: it overlaps this chunk's
            # trailing evac/normalize (DVE) instead of sitting in the tail.
            if c == 2:
                emit_oproj(0, 2)
            elif c == 3:
                emit_oproj(2, 1)

            # ---- evacuate + normalize into attnT ----
            den128 = nrmp.tile([P, CH], F32, tag="den", name=f"den{c}")
            oev = []
            for gr in range(4):
                oe = oevp.tile([64, CH], F32, tag="oev", name=f"oev{c}_{gr}")
                nc.vector.tensor_copy(oe, oa[gr][0:64, :])
                nc.vector.tensor_copy(
                    den128[ds(32 * gr, 1), :], oa[gr][64:65, :]
                )
                oev.append(oe)
            rcp128 = nrmp.tile([P, CH], F32, tag="rcp", name=f"rcp{c}")
            nc.vector.reciprocal(rcp128, den128)
            for gr in range(4):
                rch = nrmp.tile([1, CH], F32, tag="rch", name=f"rch{c}_{gr}")
                nc.vector.tensor_copy(rch, rcp128[ds(32 * gr, 1), :])
                rb = nrmp.tile([64, CH], F32, tag="rb", name=f"rb{c}_{gr}")
                nc.gpsimd.partition_broadcast(rb, rch)
                for hh in range(4):
                    h = 4 * gr + hh
                    nc.vector.tensor_mul(
                        attnT[ds(64 * (h % 2), 64), h // 2, ds(128 * c, P)],
                        oev[gr][:, ts(hh, P)],
                        rb[:, ts(hh, P)],
                    )

        emit_oproj(3, 1)

    nc.finalize()
    return nc


def _prep_inputs(x, Wq, Wkv, Wo):
    bf = ml_dtypes.bfloat16
    x = np.asarray(x, dtype=np.float32)
    Wq = np.asarray(Wq, dtype=np.float32)
    Wkv = np.asarray(Wkv, dtype=np.float32)
    Wo = np.asarray(Wo, dtype=np.float32)

    xT = [np.ascontiguousarray(x[b].T).astype(bf) for b in range(2)]
    wq = np.ascontiguousarray(Wq * SCALE).astype(bf)
    wkv = np.ascontiguousarray(Wkv).astype(bf)
    wo = np.ascontiguousarray(Wo).astype(bf)

    # triangular within-tile causal mask, replicated across the 8 head slots
    kk = np.arange(P)
    tri = (kk[:, None] <= kk[None, :]).astype(bf)          # [128 k, 128 q]
    tri8 = np.tile(tri, (1, 8))                            # [128, 1024]
    ones8 = np.ones((P, 1024), dtype=bf)

    in_maps = []
    for core in range(8):
        b, g = core // 4, core % 4
        qtiles = [4 * c + g for c in range(4)]
        xq = np.ascontiguousarray(
            np.concatenate([x[b, 128 * t : 128 * t + 128, :] for t in qtiles]).T
        ).astype(bf)
        maskq = np.concatenate(
            [tri8 if jd == g else ones8 for jd in range(4)], axis=1
        )  # [128, 4*1024]
        biasb = np.zeros((P, 4), dtype=np.float32)
        for jd in range(4):
            if jd > g:
                biasb[:, jd] = NEG
        in_maps.append(
            {
                "xT": xT[b],
                "xq": xq,
                "wq": wq,
                "wkv": wkv,
                "wo": wo,
                "maskq": np.ascontiguousarray(maskq),
                "biasb": biasb,
            }
        )
    return in_maps


def run(x, Wq, Wkv, Wo, trace=False, **trace_kwargs):
    global _PROGRAM
    if _PROGRAM is None:
        _PROGRAM = build_program()
    nc = _PROGRAM
    in_maps = _prep_inputs(x, Wq, Wkv, Wo)
    res = run_bass_kernel_spmd(
        nc, in_maps, core_ids=list(range(8)), trace=trace, **trace_kwargs
    )
    outs = res.results
    full = np.empty((2, T, HID), dtype=np.float32)
    for core in range(8):
        b, g = core // 4, core % 4
        outT = np.asarray(outs[core]["outT"]).astype(np.float32)
        for c in range(4):
            t = 4 * c + g
            full[b, 128 * t : 128 * t + 128, :] = outT[:, 128 * c : 128 * c + 128].T
    return full, res


def kernel(x, Wq, Wkv, Wo):
    out, _ = run(x, Wq, Wkv, Wo, trace=False)
    return out
